# revision 12
# baseline (speedup 1.0000x reference)
"""Dice-loss kernel for Trainium2, 8-core SPMD.

Problem: pred/label are [4,1,128,128,128] integer class maps (8 classes).
Dice needs, per batch b and class c:
    n_p[b,c] = #{pred==c},  n_l[b,c] = #{label==c},  n_i[b,c] = #{pred==c & label==c}
    score[b,c] = 2*n_i / (n_p + n_l + eps);  out[c] = mean_b score[b,c]

Sharding: core k handles batch k//2, spatial half k%2 (1,048,576 elements
per core per tensor, laid out [128, 8192] float32).

Device algorithm (exact, no per-class compare passes):
  The histogram is computed by PACKING class indicators into exponent slots
  of one fp32 value per element, then summing 63-element chunks with the
  TensorEngine.  A cubic polynomial g(v) interpolates
     g(0)=1, g(1)=2^-6, g(2)=2^-12, g(3)=2^-18, and is negative on [4,8],
  so relu(g(v)) one-hot-encodes classes 0-3 as powers of 2^-6 (exact fp32
  dyadics).  relu(g(7-v)) covers classes 4-7.  Each is ONE fused custom DVE
  instruction (8 ALU stages).  A chunk-sum of <=63 such values spans bits
  2^5..2^-18 = 24 bits, exactly an fp32 mantissa -> bit-exact counts.
  Chunk sums: one matmul per 512-column tile with a constant block-diagonal
  ones lhsT [128,3] splitting partitions into 63+63+2 rows, psum [3,512].
  Intersection streams multiply the pack by m=(pred==label) on GPSIMD.
  Engine budget/core: DVE 5 passes (~45us), GPSIMD 2, PE 96 matmuls,
  DMA 8.4MB (~24us).  Psum chunks are decoded/summed exactly on the host.
"""

import numpy as np

# ---- fixed sizes ----
NCORES = 8
P = 128
COLS = 8192            # 128*8192 = 2^20 elements per core per tensor
BLK = 2048             # columns per pipeline block
NBLK = COLS // BLK     # 4
W = 512                # matmul free dim (one psum bank)
NSUB = BLK // W        # 4
NCHUNK = NBLK * NSUB   # 16
NSTREAM = 6            # pa_p, pb_p, pa_l, pb_l, qa, qb
NC_CLASSES = 8
EPS = 1e-10

# exact cubic through (0,1),(1,2^-6),(2,2^-12),(3,2^-18); negative on [4,8]
CA = -468405.0 / 262144.0   # v coeff
CB = 504063.0 / 524288.0    # v^2 coeff
CC = -83349.0 / 524288.0    # v^3 coeff

_CACHE = {}


def _register_ops():
    """Register the two custom DVE pack ops (idempotent)."""
    from concourse import dve_ops
    from concourse.dve_spec import (
        Spec, Src0, C0, C1, C2, C3, One, relu, lower, _has_src1,
        _spill_c3_to_src1,
    )
    from concourse.dve_uop import DveOpSpec

    if "PACK_LO_DICE" in dve_ops._SUB_OPCODE_FOR_NAME:
        return (
            dve_ops.CUSTOM_DVE_SPECS["PACK_LO_DICE"]._dice_ops  # type: ignore
        )

    def horner(v):
        # relu(1 + a*v + b*v^2 + c*v^3); C0=a, C1=b, C2=c(imm)
        return relu(((C2 * v + C1) * v + C0) * v + One)

    def _np_horner(x, s0, s1, imm2):
        x = np.float32(1) * x.astype(np.float32)
        f32 = np.float32
        h = f32(f32(f32(f32(f32(imm2) * x) + f32(s1)) * x) + f32(s0)) * x
        return np.maximum(h + f32(1), f32(0))

    lo_spec = Spec(
        body=horner(Src0),
        reference=lambda in0, in1, s0, s1, imm2: _np_horner(in0, s0, s1, imm2),
    )
    hi_spec = Spec(
        body=_spill_c3_to_src1(horner(C3 - Src0)),
        reference=lambda in0, in1, s0, s1, imm2: _np_horner(
            in1.astype(np.float32) - in0.astype(np.float32), s0, s1, imm2
        ),
    )

    ops = []
    for name, spec in (("PACK_LO_DICE", lo_spec), ("PACK_HI_DICE", hi_spec)):
        row = max(dve_ops._SUB_OPCODE_FOR_NAME.values()) + 1
        assert row < 0x20
        shas = {}
        for ver in ("v3", "v4"):
            s = DveOpSpec(
                name=name, opcode=row, uops=lower(spec, ver=ver),
                rd1_en=_has_src1(spec),
            )
            shas[ver] = s.sha(ver)
        op = dve_ops.DveOp(name, spec, subdim=False, uops_sha=shas)
        dve_ops.OPS.append(op)
        dve_ops.CUSTOM_DVE_SPECS[name] = spec
        dve_ops._SUB_OPCODE_FOR_NAME[name] = row
        ops.append(op)
    lo_spec._dice_ops = tuple(ops)  # type: ignore
    return tuple(ops)


def _build_nc():
    """Build + compile the single-core Bass program (same NEFF on all cores)."""
    from contextlib import ExitStack
    import concourse.bacc as bacc
    import concourse.mybir as mybir
    import concourse.tile as tile

    pack_lo, pack_hi = _register_ops()

    f32 = mybir.dt.float32
    nc = bacc.Bacc("TRN2", target_bir_lowering=False, debug=False)

    p_d = nc.dram_tensor("p", [P, COLS], f32, kind="ExternalInput").ap()
    l_d = nc.dram_tensor("l", [P, COLS], f32, kind="ExternalInput").ap()
    w_d = nc.dram_tensor("w", [P, 32], f32, kind="ExternalInput").ap()
    # per block: 24 (stream,sub) chunk-sums -> 6 psum groups x 4 quadrant
    # bands x 3 chunk-rows
    o_d = nc.dram_tensor(
        "o", [NBLK, NSTREAM * NSUB, 3, W], f32, kind="ExternalOutput"
    ).ap()

    with ExitStack() as ctx:
        with tile.TileContext(nc) as tc:
            with (
                tc.tile_pool(name="const", bufs=1) as cpool,
                tc.tile_pool(name="io", bufs=2) as iopool,
                tc.tile_pool(name="pk", bufs=2) as pkpool,
                tc.tile_pool(name="ps", bufs=8, space="PSUM") as pspool,
            ):
                w_t = cpool.tile([P, 32], f32)
                nc.sync.dma_start(w_t[:, :], w_d)
                seven_t = cpool.tile([P, 1], f32)
                nc.vector.memset(seven_t[:, :], 7.0)

                for j in range(NBLK):
                    sl = slice(j * BLK, (j + 1) * BLK)
                    p_t = iopool.tile([P, BLK], f32, tag="p")
                    l_t = iopool.tile([P, BLK], f32, tag="l")
                    nc.sync.dma_start(p_t[:, :], p_d[:, sl])
                    nc.sync.dma_start(l_t[:, :], l_d[:, sl])

                    m_t = pkpool.tile([P, BLK], f32, tag="m")
                    nc.vector.tensor_tensor(
                        m_t[:, :], p_t[:, :], l_t[:, :],
                        mybir.AluOpType.is_equal,
                    )

                    packs = []
                    for src, lohi, tag in (
                        (p_t, 0, "pap"), (p_t, 1, "pbp"),
                        (l_t, 0, "pal"), (l_t, 1, "pbl"),
                    ):
                        t = pkpool.tile([P, BLK], f32, tag=tag)
                        if lohi == 0:
                            nc.vector._custom_dve(
                                pack_lo, out=t[:, :], in0=src[:, :],
                                s0=CA, s1=CB, imm2=CC,
                            )
                        else:
                            nc.vector._custom_dve(
                                pack_hi, out=t[:, :], in0=src[:, :],
                                in1=seven_t[:, :], s0=CA, s1=CB, imm2=CC,
                            )
                        packs.append(t)

                    qa_t = pkpool.tile([P, BLK], f32, tag="qa")
                    qb_t = pkpool.tile([P, BLK], f32, tag="qb")
                    nc.gpsimd.tensor_mul(qa_t[:, :], packs[0][:, :], m_t[:, :])
                    nc.gpsimd.tensor_mul(qb_t[:, :], packs[1][:, :], m_t[:, :])

                    # 24 chunk-sums; 3 per psum bank at partition bases 0/32/64
                    srcs = packs + [qa_t, qb_t]
                    for g in range(NSTREAM * NSUB // 3):  # 8 groups
                        ps = pspool.tile([96, W], f32, tag="ps")
                        for q in range(3):
                            gi = g * 3 + q
                            s, k = divmod(gi, NSUB)
                            nc.tensor.matmul(
                                ps[32 * q:32 * q + 32, :], lhsT=w_t[:, :],
                                rhs=srcs[s][:, k * W:(k + 1) * W],
                                start=True, stop=True,
                            )
                        st = pkpool.tile([96, W], f32, tag="st")
                        nc.scalar.copy(st[:, :], ps[:, :])
                        for q in range(3):
                            nc.sync.dma_start(
                                o_d[j, g * 3 + q], st[32 * q:32 * q + 3, :]
                            )
        nc.compile()
    return nc


def _get_nc():
    if "nc" not in _CACHE:
        _CACHE["nc"] = _build_nc()
    return _CACHE["nc"]


def _lhsT_host():
    w = np.zeros((P, 32), np.float32)
    w[0:63, 0] = 1.0
    w[63:126, 1] = 1.0
    w[126:128, 2] = 1.0
    return w


def _decode_counts(o):
    """o: [NBLK, NSTREAM*NSUB*3, W] f32 packed chunk sums -> [NSTREAM,4] int64.

    Partition rows are bands of 3 per (stream, sub); slot k (k=0..3) holds a
    count <= 63 at weight 2^(-6k)."""
    x = np.rint(o.astype(np.float64) * 262144.0).astype(np.int64)  # * 2^18
    x = x.reshape(NBLK, NSTREAM, NSUB * 3 * W)  # o is [NBLK, 24, 3, W], s-major
    cnt = np.empty((NSTREAM, 4), np.int64)
    for k in range(4):
        cnt[:, k] = ((x >> (18 - 6 * k)) & 63).sum(axis=(0, 2))
    return cnt


def kernel(pred, label):
    from concourse import bass_utils

    nc = _get_nc()
    pred = np.asarray(pred)
    label = np.asarray(label)
    w = _lhsT_host()

    in_maps = []
    for core in range(NCORES):
        b, h = core // 2, core % 2
        ps = np.ascontiguousarray(
            pred[b, 0, 64 * h:64 * (h + 1)], dtype=np.float32
        ).reshape(P, COLS)
        ls = np.ascontiguousarray(
            label[b, 0, 64 * h:64 * (h + 1)], dtype=np.float32
        ).reshape(P, COLS)
        in_maps.append({"p": ps, "l": ls, "w": w})

    res = bass_utils.run_bass_kernel_spmd(
        nc, in_maps, core_ids=list(range(NCORES)),
        trace=_CACHE.get("trace", False), **_CACHE.get("run_kwargs", {}),
    )
    _CACHE["last_res"] = res

    # n_p[b,c], n_l[b,c], n_i[b,c]
    n_p = np.zeros((4, NC_CLASSES), np.int64)
    n_l = np.zeros((4, NC_CLASSES), np.int64)
    n_i = np.zeros((4, NC_CLASSES), np.int64)
    for core in range(NCORES):
        b = core // 2
        cnt = _decode_counts(res.results[core]["o"])
        for k in range(4):
            n_p[b, k] += cnt[0, k]        # pack_lo(p): slot k <- class k
            n_p[b, 7 - k] += cnt[1, k]    # pack_hi(p): slot k <- class 7-k
            n_l[b, k] += cnt[2, k]
            n_l[b, 7 - k] += cnt[3, k]
            n_i[b, k] += cnt[4, k]
            n_i[b, 7 - k] += cnt[5, k]

    score = 2.0 * n_i / (n_p + n_l + EPS)
    return np.mean(score, axis=0).astype(np.float32)


# revision 25
# speedup vs baseline: 1.2116x; 1.2116x over previous
"""Dice-loss kernel for Trainium2, 8-core SPMD.

Problem: pred/label are [4,1,128,128,128] integer class maps (8 classes).
Dice needs, per batch b and class c:
    n_p[b,c] = #{pred==c},  n_l[b,c] = #{label==c},  n_i[b,c] = #{pred==c & label==c}
    score[b,c] = 2*n_i / (n_p + n_l + eps);  out[c] = mean_b score[b,c]

Sharding: core k handles batch k//2, spatial half k%2 (1,048,576 elements
per core per tensor, laid out [128, 8192] float32).

Device algorithm (exact, no per-class compare passes):
  Class indicators are PACKED into exponent slots of one fp16 value per
  element: a cubic g(v) interpolating
      g(0)=2^14, g(1)=2^8, g(2)=2^2, g(3)=2^-4,  g<0 on [4,8]
  makes relu(g(v)) a one-hot encoding of classes 0-3 as exact powers of
  two; the mirrored cubic covers classes 4-7.  Each is ONE fused custom
  DVE instruction (7 ALU stages), output fp16 (all values exact).
  Counts are recovered by chunk-summing with the TensorEngine: a constant
  block-diagonal lhsT [128,32] (32 chunks of 4 partition rows) and fp16
  rhs (1 col/cycle).  Per 512-col subtile, psum[32,512] accumulates 8
  matmuls (pred-pack + label-pack x 4 subtiles) -> per-slot counts <= 32,
  and sums of slot values stay exactly representable in fp32 (24-bit
  window 2^19..2^-4) -> bit-exact.  That accumulated sum is the UNION
  histogram n_p+n_l directly.  Intersection uses
      n_i = n_p + n_l - n_or,   n_or[c] = #{pred==c or label==c},
  where the or-indicator pack is just max(pack(pred), pack(label)),
  computed on GPSIMD.  Host decodes 6-bit count fields exactly.
  Engine budget/core: DVE 4 passes (~34us), GPSIMD 2, PE 24 fp16 matmul
  streams (~27us cold), ACT 16 psum copies, DMA 8.4MB in / 1MB out.
"""

import numpy as np

# ---- fixed sizes ----
NCORES = 8
P = 128
COLS = 8192            # 128*8192 = 2^20 elements per core per tensor
BLK = 1024             # columns per pipeline block
NBLK = COLS // BLK     # 8
W = 512                # matmul free dim (one psum bank)
NSUB = BLK // W        # 2
NSTREAM = 4            # u_lo, u_hi, or_lo, or_hi
NC_CLASSES = 8
EPS = 1e-10

# lo cubic: 2^14 * (1 + a v + b v^2 + c v^3); exact one-hot of classes 0-3
LO_B = (16384.0, -468405.0 / 16.0, 504063.0 / 32.0, -83349.0 / 32.0)
# hi cubic: same mirrored (v -> 7-v); one-hot of classes 4-7
HI_B = (-4961501.0 / 16.0, 6132231.0 / 32.0, -623133.0 / 16.0, 83349.0 / 32.0)

_CACHE = {}


def _register_ops():
    """Register the custom DVE pack op (idempotent).

    body = relu(((B3*v + B2)*v + B1)*v + B0)
    bindings: imm2=B3, s1=B2, s0=B1, in1=[P,1] tile holding B0 (C3 spill).
    """
    from concourse import dve_ops
    from concourse.dve_spec import (
        Spec, Src0, C0, C1, C2, C3, relu, lower, _has_src1, _spill_c3_to_src1,
    )
    from concourse.dve_uop import DveOpSpec

    if "PACK_DICE" in dve_ops._SUB_OPCODE_FOR_NAME:
        return dve_ops.CUSTOM_DVE_SPECS["PACK_DICE"]._dice_op  # type: ignore

    def _np_ref(in0, in1, s0, s1, imm2):
        f32 = np.float32
        x = in0.astype(f32)
        b0 = in1.astype(f32)  # [P,1] broadcast
        h = f32(f32(f32(f32(imm2) * x) + f32(s1)) * x + f32(s0)) * x + b0
        return np.maximum(h, f32(0))

    spec = Spec(
        body=_spill_c3_to_src1(relu(((C2 * Src0 + C1) * Src0 + C0) * Src0 + C3)),
        reference=_np_ref,
    )
    row = max(dve_ops._SUB_OPCODE_FOR_NAME.values()) + 1
    assert row < 0x20
    shas = {}
    for ver in ("v3", "v4"):
        s = DveOpSpec(
            name="PACK_DICE", opcode=row, uops=lower(spec, ver=ver),
            rd1_en=_has_src1(spec),
        )
        shas[ver] = s.sha(ver)
    op = dve_ops.DveOp("PACK_DICE", spec, subdim=False, uops_sha=shas)
    dve_ops.OPS.append(op)
    dve_ops.CUSTOM_DVE_SPECS["PACK_DICE"] = spec
    dve_ops._SUB_OPCODE_FOR_NAME["PACK_DICE"] = row
    spec._dice_op = op  # type: ignore
    return op


def _build_nc():
    """Build + compile the single-core Bass program (same NEFF on all cores)."""
    import concourse.bacc as bacc
    import concourse.mybir as mybir
    import concourse.tile as tile

    pack_op = _register_ops()

    f32 = mybir.dt.float32
    f16 = mybir.dt.float16
    nc = bacc.Bacc("TRN2", target_bir_lowering=False, debug=False)

    p_d = nc.dram_tensor("p", [P, COLS], f16, kind="ExternalInput").ap()
    l_d = nc.dram_tensor("l", [P, COLS], f16, kind="ExternalInput").ap()
    w_d = nc.dram_tensor("w", [P, P], f16, kind="ExternalInput").ap()
    o_d = nc.dram_tensor(
        "o", [NSTREAM, P, W], f32, kind="ExternalOutput"
    ).ap()

    with tile.TileContext(nc) as tc:
        with (
            tc.tile_pool(name="const", bufs=1) as cpool,
            tc.tile_pool(name="io", bufs=3) as iopool,
            tc.tile_pool(name="pk", bufs=3) as pkpool,
            tc.tile_pool(name="ps", bufs=1, space="PSUM") as pspool,
        ):
            # block-0 input DMAs first so the DVE can start ASAP; the
            # weight tile is only needed by the first matmul, later.
            io_tiles = []
            for j in range(NBLK):
                sl = slice(j * BLK, (j + 1) * BLK)
                p_t = iopool.tile([P, BLK], f16, tag="p", name=f"p_t{j}")
                l_t = iopool.tile([P, BLK], f16, tag="l", name=f"l_t{j}")
                if j == 0:
                    nc.sync.dma_start(p_t[:, :], p_d[:, sl])
                    nc.sync.dma_start(l_t[:, :], l_d[:, sl])
                io_tiles.append((p_t, l_t))

            w_t = cpool.tile([P, P], f16)
            nc.sync.dma_start(w_t[:, :], w_d)
            b0lo_t = cpool.tile([P, 1], f32)
            nc.vector.memset(b0lo_t[:, :], LO_B[0])
            b0hi_t = cpool.tile([P, 1], f32)
            nc.vector.memset(b0hi_t[:, :], HI_B[0])

            # psum accumulators live across all blocks
            ps_tiles = [
                pspool.tile([P, W], f32, tag=f"ps{s}", name=f"ps{s}")
                for s in range(NSTREAM)
            ]
            for j in range(NBLK):
                sl = slice(j * BLK, (j + 1) * BLK)
                p_t, l_t = io_tiles[j]
                if j > 0:
                    nc.sync.dma_start(p_t[:, :], p_d[:, sl])
                    nc.scalar.dma_start(l_t[:, :], l_d[:, sl])

                packs = {}
                for src, nm, lohi in (
                    (p_t, "pap", 0), (p_t, "pbp", 1),
                    (l_t, "pal", 0), (l_t, "pbl", 1),
                ):
                    t = pkpool.tile([P, BLK], f16, tag=nm)
                    coef = LO_B if lohi == 0 else HI_B
                    b0t = b0lo_t if lohi == 0 else b0hi_t
                    nc.vector._custom_dve(
                        pack_op, out=t[:, :], in0=src[:, :], in1=b0t[:, :],
                        s0=coef[1], s1=coef[2], imm2=coef[3],
                    )
                    packs[nm] = t

                d_t = pkpool.tile([P, BLK], f16, tag="d")
                nc.gpsimd.tensor_sub(d_t[:, :], p_t[:, :], l_t[:, :])
                m_t = pkpool.tile([P, BLK], f16, tag="m")
                nc.gpsimd.tensor_scalar(
                    m_t[:, :], d_t[:, :], 0.0, None,
                    mybir.AluOpType.is_equal)
                qa_t = pkpool.tile([P, BLK], f16, tag="qa")
                qb_t = pkpool.tile([P, BLK], f16, tag="qb")
                nc.gpsimd.tensor_mul(qa_t[:, :], packs["pap"][:, :], m_t[:, :])
                nc.gpsimd.tensor_mul(qb_t[:, :], packs["pbp"][:, :], m_t[:, :])

                # 4 output streams; psum [P, W] accumulates across blocks
                streams = [
                    (packs["pap"], packs["pal"]),   # u_lo
                    (packs["pbp"], packs["pbl"]),   # u_hi
                    (qa_t,),                        # i_lo
                    (qb_t,),                        # i_hi
                ]
                for s, srcs in enumerate(streams):
                    n_mm = len(srcs) * NSUB
                    i = 0
                    for src in srcs:
                        for k in range(NSUB):
                            nc.tensor.matmul(
                                ps_tiles[s][:, :], lhsT=w_t[:, :],
                                rhs=src[:, k * W:(k + 1) * W],
                                start=(j == 0 and i == 0),
                                stop=(j == NBLK - 1 and i == n_mm - 1),
                            )
                            i += 1

            for s in range(NSTREAM):
                st = pkpool.tile([P, W], f32, tag=f"st{s}", name=f"st{s}")
                if s < 2:   # u streams finish first; DVE is idle by then
                    nc.vector.tensor_copy(st[:, :], ps_tiles[s][:, :])
                    nc.scalar.dma_start(o_d[s], st[:, :])
                else:
                    nc.scalar.copy(st[:, :], ps_tiles[s][:, :])
                    nc.sync.dma_start(o_d[s], st[:, :])
    nc.compile()
    return nc


def _get_nc():
    if "nc" not in _CACHE:
        _CACHE["nc"] = _build_nc()
    return _CACHE["nc"]


def _lhsT_host():
    return np.eye(P, dtype=np.float16)


def _decode_counts(o):
    """o: [NBLK, NSTREAM, 32, W] f32 packed chunk sums -> [NSTREAM, 4] int64.

    value = sum_k cnt_k * 2^(14-6k), cnt_k <= 32; scale by 2^4 -> 6-bit
    fields at bits 18/12/6/0."""
    x = np.rint(o.astype(np.float64) * 16.0).astype(np.int64)
    x = x.reshape(NSTREAM, P * W)
    cnt = np.empty((NSTREAM, 4), np.int64)
    for k in range(4):
        cnt[:, k] = ((x >> (18 - 6 * k)) & 63).sum(axis=1)
    return cnt


def kernel(pred, label):
    from concourse import bass_utils

    nc = _get_nc()
    pred = np.asarray(pred)
    label = np.asarray(label)
    w = _lhsT_host()

    in_maps = []
    for core in range(NCORES):
        b, h = core // 2, core % 2
        ps = np.ascontiguousarray(
            pred[b, 0, 64 * h:64 * (h + 1)], dtype=np.float16
        ).reshape(P, COLS)
        ls = np.ascontiguousarray(
            label[b, 0, 64 * h:64 * (h + 1)], dtype=np.float16
        ).reshape(P, COLS)
        in_maps.append({"p": ps, "l": ls, "w": w})

    res = bass_utils.run_bass_kernel_spmd(
        nc, in_maps, core_ids=list(range(NCORES)),
        trace=_CACHE.get("trace", False), **_CACHE.get("run_kwargs", {}),
    )
    _CACHE["last_res"] = res

    n_u = np.zeros((4, NC_CLASSES), np.int64)   # n_p + n_l
    n_i = np.zeros((4, NC_CLASSES), np.int64)   # intersection
    for core in range(NCORES):
        b = core // 2
        cnt = _decode_counts(res.results[core]["o"])
        for k in range(4):
            n_u[b, k] += cnt[0, k]        # u_lo: slot k <- class k
            n_u[b, 7 - k] += cnt[1, k]    # u_hi: slot k <- class 7-k
            n_i[b, k] += cnt[2, k]
            n_i[b, 7 - k] += cnt[3, k]

    score = 2.0 * n_i / (n_u + EPS)
    return np.mean(score, axis=0).astype(np.float32)
